# revision 1
# baseline (speedup 1.0000x reference)
"""Trainium2 Bass kernel for nn_NestedFormula.

Tree: DEPTH=4, V=4. Level sizes n4=1, n3=5, n2=25, n1=125, n0=125.
  f1[n] = sum_v lam1[n,v] * x_v^pow1[n,v] + lam0[n]
  fd[n] = sum_v lamd[n,v] * x_v^powd[n,v] * f_{d-1}[5n+v] + f_{d-1}[5n+4]
  out   = f4[0]                          (per batch element)

Strategy (pure data parallel over batch, 8 cores x 16384):
  - x^p = exp(p * ln x): one packed Ln + per-partition-scaled Exp calls on the
    scalar engine (the bottleneck: ~500+100+20+4 exps per batch elem).
  - (node,var) pairs live on partitions; batch on the free dim in 4 chunks
    of 4096. Levels 3/4 pack the chunk index into partitions too.
  - All weighted reductions are matmuls with host-precomputed block-diagonal
    G matrices (float32r for full PE rate). Gather patterns, last-subformula
    passthroughs (exp(0)=1 rows with weight-1 columns) and the lam0 bias
    (ones-row in the last L1 tile) are all folded into the G columns.
  - ln(x) is computed once packed (128,512), bounced to DRAM, and
    broadcast-read into replicated layouts with step-0 DMA access patterns.
"""
import numpy as np

import concourse.bacc as bacc
import concourse.mybir as mybir
from concourse.tile import TileContext

DEPTH = 4
V = 4
B = 131072
M_CORES = 8
BS = B // M_CORES          # 16384 per core
CHUNK = 4096
NCH = BS // CHUNK          # 4
HALF = 2048                # psum granularity
MMN = 512                  # matmul free dim (one PSUM bank)

F32 = mybir.dt.float32
F32R = mybir.dt.float32r

N1, N2, N3, N4 = 125, 25, 5, 1
J1 = 4 * N1                # 500 level-1 (node,var) pairs
NT1 = 4                    # level-1 j-tiles of 128


def _sigma1(m):
    # psum1 row m -> level-1 node index
    if m < 100:
        return 5 * (m // 4) + (m % 4)
    return 5 * (m - 100) + 4


def _tau2(m):
    # psum2 row m -> level-2 node index
    if m < 20:
        return 5 * (m // 4) + (m % 4)
    return 5 * (m - 20) + 4


def build_constants(lam0, lam1, pow1, lam2, pow2, lam3, pow3, lam4, pow4):
    c = {}
    # ---- level 1: 4 j-tiles of K=128, M=128 (125 used cols) ----
    sc1 = np.zeros((128, NT1), np.float32)
    g1 = np.zeros((NT1, 128, 128), np.float32)
    for n in range(N1):
        for v in range(V):
            j = 4 * n + v
            t, r = divmod(j, 128)
            sc1[r, t] = pow1[n, v]
    for m in range(125):
        n = _sigma1(m)
        for v in range(V):
            j = 4 * n + v
            t, r = divmod(j, 128)
            g1[t, r, m] = lam1[n, v]
        g1[3, 116, m] = lam0[n]          # ones-row (sc1[116,3]=0 -> exp=1)
    c["sc1"] = sc1
    c["g1"] = np.ascontiguousarray(g1.transpose(1, 0, 2).reshape(128, NT1 * 128))

    # ---- level 2: K=128 (100 exp rows + 25 passthrough), M=32 (25 used) ----
    sc2 = np.zeros((128, 1), np.float32)
    g2 = np.zeros((128, 32), np.float32)
    for n in range(N2):
        for v in range(V):
            sc2[4 * n + v, 0] = pow2[n, v]
    for m in range(25):
        n2t = _tau2(m)
        for v in range(V):
            g2[4 * n2t + v, m] = lam2[n2t, v]
        g2[100 + n2t, m] = 1.0           # + f1[5*n2t+4] passthrough
    c["sc2"] = sc2
    c["g2"] = g2

    # ---- level 3 (chunk-packed): rows 32c+m2, cols 5c+u ----
    sc3 = np.zeros((128, 1), np.float32)
    g3 = np.zeros((128, 32), np.float32)
    for cc in range(NCH):
        for m2 in range(25):
            r = 32 * cc + m2
            if m2 < 20:
                n3, v3 = divmod(m2, 4)
                sc3[r, 0] = pow3[n3, v3]
                g3[r, 5 * cc + n3] = lam3[n3, v3]
            else:
                g3[r, 5 * cc + (m2 - 20)] = 1.0   # + f2[5*n3+4]
    c["sc3"] = sc3
    c["g3"] = g3

    # ---- level 4 (chunk-packed): rows 5c+u, cols c ----
    sc4 = np.zeros((32, 1), np.float32)
    g4 = np.zeros((32, NCH), np.float32)
    for cc in range(NCH):
        for u in range(4):
            sc4[5 * cc + u, 0] = pow4[0, u]
            g4[5 * cc + u, cc] = lam4[0, u]
        g4[5 * cc + 4, cc] = 1.0                  # + f3[4]
    c["sc4"] = sc4
    c["g4"] = g4
    return c


def build_bass():
    nc = bacc.Bacc()
    xt = nc.dram_tensor("xt", (V, BS), F32, kind="ExternalInput")
    sc1 = nc.dram_tensor("sc1", (128, NT1), F32, kind="ExternalInput")
    g1 = nc.dram_tensor("g1", (128, NT1 * 128), F32R, kind="ExternalInput")
    sc2 = nc.dram_tensor("sc2", (128, 1), F32, kind="ExternalInput")
    g2 = nc.dram_tensor("g2", (128, 32), F32R, kind="ExternalInput")
    sc3 = nc.dram_tensor("sc3", (128, 1), F32, kind="ExternalInput")
    g3 = nc.dram_tensor("g3", (128, 32), F32R, kind="ExternalInput")
    sc4 = nc.dram_tensor("sc4", (32, 1), F32, kind="ExternalInput")
    g4 = nc.dram_tensor("g4", (32, NCH), F32R, kind="ExternalInput")
    y = nc.dram_tensor("y", (BS,), F32, kind="ExternalOutput")

    EXP = mybir.ActivationFunctionType.Exp
    LN = mybir.ActivationFunctionType.Ln

    with TileContext(nc) as tc:
        with tc.tile_pool(name="const", bufs=1) as cpool, \
             tc.tile_pool(name="dram", bufs=1, space="DRAM") as dpool, \
             tc.tile_pool(name="big", bufs=1) as bpool, \
             tc.tile_pool(name="psum", bufs=2, space="PSUM") as ppool:
            # one shared pool of 16KB/partition slots; phase-B tiles reuse
            # phase-A slots via tags (disjoint lifetimes)
            lpool = e1pool = e2pool = spool = bpool

            # ---------- constants into SBUF (G's cast to f32r) ----------
            sct1 = cpool.tile([128, NT1], F32)
            nc.sync.dma_start(out=sct1[:], in_=sc1[:, :])
            sct2 = cpool.tile([128, 1], F32)
            nc.sync.dma_start(out=sct2[:], in_=sc2[:, :])
            sct3 = cpool.tile([128, 1], F32)
            nc.sync.dma_start(out=sct3[:], in_=sc3[:, :])
            sct4 = cpool.tile([32, 1], F32)
            nc.sync.dma_start(out=sct4[:], in_=sc4[:, :])

            def load_g(dram_t, shape, tag):
                r = cpool.tile(list(shape), F32R, tag=tag)
                nc.sync.dma_start(out=r[:], in_=dram_t[:, :])
                return r

            g1t = load_g(g1, (128, NT1 * 128), "g1t")
            g2t = load_g(g2, (128, 32), "g2t")
            g3t = load_g(g3, (128, 32), "g3t")
            g4t = load_g(g4, (32, NCH), "g4t")

            # ---------- ln(x) packed, bounce to DRAM (per-chunk pipelined) ----
            # xc rows 4c''+v = x[v, 512c'':512(c''+1)]; chunk c = rows 32c..
            xc = cpool.tile([128, 512], F32, tag="xc")
            lc = cpool.tile([128, 512], F32, tag="lc")
            # per-chunk DRAM scratch tiles for ln(x) -> exact DMA deps
            lds = []
            for cc in range(NCH):
                r0 = 32 * cc
                xt_view = xt[:, cc * CHUNK:(cc + 1) * CHUNK] \
                    .rearrange("v (c i) -> c v i", i=512)
                nc.sync.dma_start(out=xc[r0:r0 + 32, :], in_=xt_view)
                nc.scalar.activation(lc[r0:r0 + 32, :], xc[r0:r0 + 32, :], LN)
                ldc = dpool.tile([V, CHUNK], F32, tag=f"ld{cc}")
                nc.sync.dma_start(
                    out=ldc[:, :].rearrange("v (c i) -> c v i", i=512),
                    in_=lc[r0:r0 + 32, :])
                lds.append(ldc)


            # f2all rows 32c+m2 = f2-stage values of chunk c
            f2all = spool.tile([128, CHUNK], F32, tag="f2all", bufs=1)

            # ---------- phase A: levels 1-2, per chunk ----------
            for cc in range(NCH):
                col0 = cc * CHUNK
                lrep = lpool.tile([128, CHUNK], F32, tag="lrep", bufs=2)
                nc.sync.dma_start(
                    out=lrep[:],
                    in_=lds[cc][:, :].unsqueeze(0)
                        .broadcast_to([32, V, CHUNK]))

                e2 = e2pool.tile([128, CHUNK], F32R, tag="e2", bufs=2)
                nc.scalar.activation(e2[:], lrep[:], EXP, scale=sct2[:, 0:1])

                for h in range(CHUNK // HALF):
                    hc = h * HALF
                    e1s = []
                    for t in range(NT1):
                        e1 = e1pool.tile([128, HALF], F32R, tag="e1", bufs=5)
                        nc.scalar.activation(e1[:], lrep[:, hc:hc + HALF],
                                             EXP, scale=sct1[:, t:t + 1])
                        e1s.append(e1)
                    ps1 = ppool.tile([128, HALF], F32, tag="ps")
                    for s in range(HALF // MMN):
                        scol = s * MMN
                        for t in range(NT1):
                            nc.tensor.matmul(
                                ps1[:, scol:scol + MMN],
                                g1t[:, 128 * t:128 * (t + 1)],
                                e1s[t][:, scol:scol + MMN],
                                start=(t == 0), stop=(t == NT1 - 1))
                    # X2 = E2x * psum1 (in place into e2)
                    nc.vector.tensor_mul(
                        e2[:, hc:hc + HALF], e2[:, hc:hc + HALF],
                        ps1[:].bitcast(F32R))
                    ps2 = ppool.tile([32, HALF], F32, tag="ps")
                    for s in range(HALF // MMN):
                        scol = s * MMN
                        nc.tensor.matmul(
                            ps2[:, scol:scol + MMN], g2t[:],
                            e2[:, hc + scol:hc + scol + MMN],
                            start=True, stop=True)
                    # drain f2 (chunk-packed rows 32c+m2)
                    nc.vector.tensor_copy(
                        f2all[32 * cc:32 * cc + 32, hc:hc + HALF], ps2[:])

            # ---------- phase B: levels 3-4, chunk-packed ----------
            # phase-B exp inputs (depend only on ld)
            l3x = spool.tile([128, CHUNK], F32R, tag="l3x", bufs=1)
            for cc in range(NCH):
                col0 = cc * CHUNK
                nc.sync.dma_start(
                    out=l3x[32 * cc:32 * (cc + 1), :].bitcast(F32),
                    in_=lds[cc][:, :].unsqueeze(0)
                        .broadcast_to([8, V, CHUNK]))
            nc.scalar.activation(l3x[:], l3x[:].bitcast(F32), EXP,
                                 scale=sct3[:, 0:1])

            l4x = spool.tile([32, CHUNK], F32R, tag="l4x", bufs=1)
            for cc in range(NCH):
                col0 = cc * CHUNK
                nc.sync.dma_start(out=l4x[5 * cc:5 * cc + 4, :].bitcast(F32),
                                  in_=lds[cc][:, :])
                nc.sync.dma_start(
                    out=l4x[5 * cc + 4:5 * cc + 5, :].bitcast(F32),
                    in_=lds[cc][0:1, :])
            # rows 20..31 only need finite values (scale=0): reuse ld chunk 0
            nc.sync.dma_start(
                out=l4x[20:32, :].bitcast(F32),
                in_=lds[0][:, :].unsqueeze(0).broadcast_to([3, V, CHUNK]))
            nc.scalar.activation(l4x[:], l4x[:].bitcast(F32), EXP,
                                 scale=sct4[:, 0:1])


            for h in range(CHUNK // HALF):
                hc = h * HALF
                # X3 = E3x * f2all (in place, per half; waits on the last
                # chunk's h-drain only)
                nc.vector.tensor_mul(l3x[:, hc:hc + HALF], l3x[:, hc:hc + HALF],
                                     f2all[:, hc:hc + HALF].bitcast(F32R))
                ps3 = ppool.tile([32, HALF], F32, tag="ps")
                for s in range(HALF // MMN):
                    scol = s * MMN
                    nc.tensor.matmul(ps3[:, scol:scol + MMN], g3t[:],
                                     l3x[:, hc + scol:hc + scol + MMN],
                                     start=True, stop=True)
                # X4 = E4x * psum3 (in place into l4x half)
                nc.vector.tensor_mul(l4x[:, hc:hc + HALF],
                                     l4x[:, hc:hc + HALF],
                                     ps3[:].bitcast(F32R))
                ps4 = ppool.tile([NCH, HALF], F32, tag="ps")
                for s in range(HALF // MMN):
                    scol = s * MMN
                    nc.tensor.matmul(ps4[:, scol:scol + MMN], g4t[:],
                                     l4x[:, hc + scol:hc + scol + MMN],
                                     start=True, stop=True)
                outsb = spool.tile([NCH, HALF], F32, tag="outsb", bufs=2)
                nc.vector.tensor_copy(outsb[:], ps4[:])
                nc.sync.dma_start(
                    out=y[:].rearrange("(c i) -> c i", i=CHUNK)[:, hc:hc + HALF],
                    in_=outsb[:])

    nc.compile()
    return nc


def kernel(x, lam0, lam1, pow1, lam2, pow2, lam3, pow3, lam4, pow4):
    x = np.asarray(x, np.float32)
    consts = build_constants(
        np.asarray(lam0, np.float32), np.asarray(lam1, np.float32),
        np.asarray(pow1, np.float32), np.asarray(lam2, np.float32),
        np.asarray(pow2, np.float32), np.asarray(lam3, np.float32),
        np.asarray(pow3, np.float32), np.asarray(lam4, np.float32),
        np.asarray(pow4, np.float32))

    nc = build_bass()

    in_maps = []
    for k in range(M_CORES):
        shard = x[k * BS:(k + 1) * BS, :]
        m = {"xt": np.ascontiguousarray(shard.T)}
        m.update(consts)
        in_maps.append(m)

    from concourse.bass_utils import run_bass_kernel_spmd
    res = run_bass_kernel_spmd(nc, in_maps, list(range(M_CORES)))
    out = np.concatenate([res.results[k]["y"] for k in range(M_CORES)])
    return out[:, None].astype(np.float32)


if __name__ == "__main__":
    import reference
    inputs = {k: np.asarray(v) for k, v in reference.setup_inputs().items()}
    got = kernel(**inputs)
    exp = np.asarray(reference.reference(**inputs))
    err = np.abs(got - exp).max() / (np.abs(exp).max() + 1e-30)
    print("shape", got.shape, "relerr", err)

